# revision 1
# baseline (speedup 1.0000x reference)
"""Single-head cross-attention kernel for Trainium2, sharded across 8 NeuronCores.

Strategy (per core c):
  - query shard: x_1 rows [512c, 512c+512); key/value shard: x_2 same slice.
  - Split activations x into fp16 hi + bf16 lo halves, DMA-transpose the 2-byte
    halves (xbar) to get xT layout needed by the PE (contraction on partitions).
  - Projections as 2-pass matmuls (hi@W_f16 + lo@W_bf16, fp32 PSUM accumulate)
    producing transposed outputs QT/KT [d, seq]; V in natural layout [seq, d]
    (single fp16 pass - V precision is uncritical).
  - AllGather the KT/V shards (fp16, 2MB/core) across the 8 cores.
  - Scores computed TRANSPOSED: ST[keys, q] = KT_full.T-contracted @ QT, fp16
    operands, fp32 PSUM. Softmax max is reduced on DVE across key tiles, then
    across partitions via PE transpose; broadcast back with a rank-1 matmul.
  - P^T = exp((ST - max)/32) in fp16 is directly the lhsT for the AV matmul;
    row sums ride along as an extra N=1 matmul against a ones vector.
  - Output O[q, d] = (P^T.T @ V) scaled by 1/rowsum on PSUM eviction.

Numerics (validated against fp64 on host): rel err ~1.1e-3 end to end; the
softmax here is nearly one-hot (score std ~8000 post-scale) so score-path
precision is held at >=fp16-operand/fp32-accumulate everywhere.
"""
import numpy as np

import concourse.bacc as bacc
import concourse.mybir as mybir
import concourse.tile as tile
from concourse.bass_utils import run_bass_kernel_spmd
from concourse.masks import make_identity

P = 128
D = 1024            # d_in = d_kq = d_v
DP = D // P         # 8 partition tiles of the feature dim
S = 4096            # full sequence length (both x_1 and x_2)
NCORES = 8
SQ = S // NCORES    # 512 query rows per core
SK = S // NCORES    # 512 key rows per core
NH = 2              # process queries in halves for SBUF + pipelining
QH = SQ // NH       # 256
NKT = S // P        # 32 key tiles of 128
SCALE = float(1.0 / np.sqrt(np.float32(D)))  # 0.03125 exactly

F32 = mybir.dt.float32
F16 = mybir.dt.float16
BF16 = mybir.dt.bfloat16
AX = mybir.AxisListType
AF = mybir.ActivationFunctionType

_CACHED_NC = None


def _split_transpose(nc, sb, dram, x_ap, rows, name):
    """Split fp32 x [rows, D] into f16 hi + bf16 lo and return the transposed
    tiles xT_hi[d], xT_lo[d] (each [P, rows]) via a DRAM round trip through the
    2-byte xbar DMA-transpose."""
    hi_d = dram.tile([rows, D], F16, name=f"{name}_hi_d")
    lo_d = dram.tile([rows, D], BF16, name=f"{name}_lo_d")
    for m in range(rows // P):
        xf = sb.tile([P, D], F32, tag="xf", bufs=4, name=f"{name}_xf{m}")
        nc.sync.dma_start(xf, x_ap[m * P:(m + 1) * P, :])
        hi = sb.tile([P, D], F16, tag="xhi", bufs=4, name=f"{name}_hi{m}")
        nc.scalar.copy(hi, xf)
        lo = sb.tile([P, D], BF16, tag="xlo", bufs=4, name=f"{name}_lo{m}")
        nc.vector.tensor_sub(lo, xf, hi)
        nc.sync.dma_start(hi_d[m * P:(m + 1) * P, :], hi)
        nc.sync.dma_start(lo_d[m * P:(m + 1) * P, :], lo)
    t_hi, t_lo = [], []
    for d in range(DP):
        th = sb.tile([P, rows], F16, tag=f"{name}_th", bufs=DP, name=f"{name}_th{d}")
        nc.sync.dma_start(th, hi_d[:, d * P:(d + 1) * P], transpose=True)
        tl = sb.tile([P, rows], BF16, tag=f"{name}_tl", bufs=DP, name=f"{name}_tl{d}")
        nc.sync.dma_start(tl, lo_d[:, d * P:(d + 1) * P], transpose=True)
        t_hi.append(th)
        t_lo.append(tl)
    return t_hi, t_lo


def build_nc():
    nc = bacc.Bacc("TRN2", target_bir_lowering=False, debug=False,
                   num_devices=NCORES)
    x1 = nc.dram_tensor("x1s", [SQ, D], F32, kind="ExternalInput").ap()
    x2 = nc.dram_tensor("x2s", [SK, D], F32, kind="ExternalInput").ap()
    wq = nc.dram_tensor("wq", [D, D], F32, kind="ExternalInput").ap()
    wk = nc.dram_tensor("wk", [D, D], F32, kind="ExternalInput").ap()
    wv = nc.dram_tensor("wv", [D, D], F32, kind="ExternalInput").ap()
    out = nc.dram_tensor("out", [SQ, D], F32, kind="ExternalOutput").ap()

    with tile.TileContext(nc) as tc:
        with tc.tile_pool(name="long", bufs=1) as long_pool, \
             tc.tile_pool(name="dram", bufs=1, space="DRAM") as dram:
            # long-lived constants + QT
            ident = long_pool.tile([P, P], F32, name="ident")
            make_identity(nc, ident)
            ones1 = long_pool.tile([1, P], F32, name="ones1")
            nc.vector.memset(ones1, 1.0)
            ones16 = long_pool.tile([P, 1], F16, name="ones16")
            nc.vector.memset(ones16, 1.0)

            ag_in_k = dram.tile([DP, P, SK], F16, name="ag_in_k")
            ag_out_k = dram.tile([NCORES, DP, P, SK], F16,
                                 addr_space="Shared", name="ag_out_k")
            ag_in_v = dram.tile([DP, P, SK], F16, name="ag_in_v")
            ag_out_v = dram.tile([NCORES, DP, P, SK], F16,
                                 addr_space="Shared", name="ag_out_v")

            qt16 = [long_pool.tile([P, SQ], F16, name=f"qt16_{d}")
                    for d in range(DP)]

            with tc.tile_pool(name="wpool", bufs=1) as wp, \
                 tc.tile_pool(name="proj_ps", bufs=1, space="PSUM") as pps:
                # x splits + transposes (x2 first: the K/V side gates the AG)
                with tc.tile_pool(name="splits", bufs=1) as sp:
                    x2t_hi, x2t_lo = _split_transpose(nc, sp, dram, x2, SK, "x2")

                    # weights via cast-DMA (SWDGE queue, parallel with the
                    # sync-queue x chain); K first - it gates AG-K
                    wk16 = wp.tile([P, DP, D], F16, name="wk16")
                    nc.gpsimd.dma_start(wk16, wk.rearrange("(dp p) n -> p dp n", p=P))
                    wkbf = wp.tile([P, DP, D], BF16, name="wkbf")
                    nc.gpsimd.dma_start(wkbf, wk.rearrange("(dp p) n -> p dp n", p=P))
                    wv16 = wp.tile([P, DP, D], F16, name="wv16")
                    nc.gpsimd.dma_start(wv16, wv.rearrange("(dp p) n -> p dp n", p=P))
                    wq16 = wp.tile([P, DP, D], F16, name="wq16")
                    nc.gpsimd.dma_start(wq16, wq.rearrange("(dp p) n -> p dp n", p=P))
                    wqbf = wp.tile([P, DP, D], BF16, name="wqbf")
                    nc.gpsimd.dma_start(wqbf, wq.rearrange("(dp p) n -> p dp n", p=P))

                    # PE warm-up: HAM un-throttles after ~3.4us of activity.
                    # These depend on the first x2 transpose, so they run just
                    # before the real projections instead of at t=0.
                    dummy16 = long_pool.tile([P, P], F16, name="dummy16")
                    nc.vector.memset(dummy16, 0.0)
                    for w in range(24):
                        wps = pps.tile([P, 512], F32, tag="pp", bufs=4,
                                       name=f"warm{w}")
                        nc.tensor.matmul(wps, lhsT=dummy16,
                                         rhs=x2t_hi[0][:, 0:512],
                                         start=True, stop=True)

                    # KT projection: KT[d_out] = Wk.T @ x2^T  [P, SK]
                    for do in range(DP):
                        ps = pps.tile([P, SK], F32, tag="pp", bufs=4, name=f"ktps{do}")
                        cs = slice(do * P, (do + 1) * P)
                        for ki in range(DP):
                            nc.tensor.matmul(ps, lhsT=wk16[:, ki, cs],
                                             rhs=x2t_hi[ki],
                                             start=(ki == 0), stop=False)
                        for ki in range(DP):
                            nc.tensor.matmul(ps, lhsT=wkbf[:, ki, cs],
                                             rhs=x2t_lo[ki],
                                             start=False, stop=(ki == DP - 1))
                        kt_t = sp.tile([P, SK], F16, tag="kt16", bufs=3,
                                       name=f"kt16_{do}")
                        nc.scalar.copy(kt_t, ps)
                        nc.sync.dma_start(ag_in_k[do], kt_t)

                    # AG-K dispatched early: overlaps V + QT projections
                    nc.gpsimd.collective_compute(
                        "AllGather", mybir.AluOpType.bypass,
                        replica_groups=[list(range(NCORES))],
                        ins=[ag_in_k.opt()], outs=[ag_out_k.opt()])

                    # V projection: V[kt block] = x2 @ Wv  [P keys, D], fp16 1-pass
                    for kt in range(SK // P):
                        for dvc in range(2):
                            ps = pps.tile([P, 512], F32, tag="pp", bufs=4,
                                          name=f"vps{kt}_{dvc}")
                            ds_ = slice(dvc * 512, (dvc + 1) * 512)
                            for ki in range(DP):
                                nc.tensor.matmul(
                                    ps, lhsT=x2t_hi[ki][:, kt * P:(kt + 1) * P],
                                    rhs=wv16[:, ki, ds_],
                                    start=(ki == 0), stop=(ki == DP - 1))
                            v_t = sp.tile([P, 512], F16, tag="v16", bufs=3,
                                          name=f"v16_{kt}_{dvc}")
                            nc.scalar.copy(v_t, ps)
                            nc.sync.dma_start(ag_in_v[2 * kt + dvc], v_t)

                    # AG-V: not needed until the AV phase, ~150us later
                    nc.gpsimd.collective_compute(
                        "AllGather", mybir.AluOpType.bypass,
                        replica_groups=[list(range(NCORES))],
                        ins=[ag_in_v.opt()], outs=[ag_out_v.opt()])

                    # x1 chain now: its DMAs no longer compete with x2/W
                    x1t_hi, x1t_lo = _split_transpose(nc, sp, dram, x1, SQ, "x1")

                    # QT projection
                    for do in range(DP):
                        ps = pps.tile([P, SQ], F32, tag="pp", bufs=4, name=f"qtps{do}")
                        cs = slice(do * P, (do + 1) * P)
                        for ki in range(DP):
                            nc.tensor.matmul(ps, lhsT=wq16[:, ki, cs],
                                             rhs=x1t_hi[ki],
                                             start=(ki == 0), stop=False)
                        for ki in range(DP):
                            nc.tensor.matmul(ps, lhsT=wqbf[:, ki, cs],
                                             rhs=x1t_lo[ki],
                                             start=False, stop=(ki == DP - 1))
                        nc.scalar.copy(qt16[do], ps)

            # ---- attention: scores -> softmax -> AV, in query halves ----
            with tc.tile_pool(name="attn", bufs=1) as ap_, \
                 tc.tile_pool(name="attn_ps", bufs=1, space="PSUM") as aps:
                st_tiles = [[None] * NKT for _ in range(NH)]
                pt_tiles = [[None] * NKT for _ in range(NH)]
                m1 = [None] * NH
                mb = [None] * NH

                def scores(h):
                    qsl = slice(h * QH, (h + 1) * QH)
                    ktg = None
                    for kt in range(NKT):
                        r, k = divmod(kt, SK // P)
                        if k == 0:
                            # one batched 1MB load per rank block
                            ktg = ap_.tile([P, DP, SK], F16, tag="ktg", bufs=3,
                                           name=f"ktg{h}_{r}")
                            nc.sync.dma_start(
                                ktg, ag_out_k[r].rearrange("d p s -> p d s"))
                        ps = aps.tile([P, QH], F32, tag="sc", bufs=2,
                                      name=f"stps{h}_{kt}")
                        for d in range(DP):
                            nc.tensor.matmul(
                                ps, lhsT=ktg[:, d, k * P:(k + 1) * P],
                                rhs=qt16[d][:, qsl],
                                start=(d == 0), stop=(d == DP - 1))
                        st = ap_.tile([P, QH], F32, tag="st", bufs=44,
                                      name=f"st{h}_{kt}")
                        nc.vector.tensor_copy(st, ps)
                        st_tiles[h][kt] = st
                        mn = ap_.tile([P, QH], F32, tag="m1", bufs=3,
                                      name=f"m1_{h}_{kt}")
                        if kt == 0:
                            nc.vector.tensor_copy(mn, st)
                        else:
                            nc.vector.tensor_max(mn, m1[h], st)
                        m1[h] = mn

                def soft_prep(h):
                    # cross-partition max: PE-transpose m1 128-blocks, DVE reduce
                    mrow = ap_.tile([1, QH], F32, tag="mrow", bufs=2,
                                    name=f"mrow{h}")
                    for b in range(QH // P):
                        tps = aps.tile([P, P], F32, tag="sc", bufs=2,
                                       name=f"tps{h}_{b}")
                        nc.tensor.transpose(tps, m1[h][:, b * P:(b + 1) * P], ident)
                        mq = ap_.tile([P, 1], F32, tag="mq", bufs=2,
                                      name=f"mq{h}_{b}")
                        nc.vector.reduce_max(mq, tps, axis=AX.X)
                        rps = aps.tile([1, P], F32, tag="sc", bufs=2,
                                       name=f"rps{h}_{b}")
                        nc.tensor.transpose(rps, mq, ident)
                        nc.vector.tensor_copy(mrow[:, b * P:(b + 1) * P], rps)
                    mbps = aps.tile([P, QH], F32, tag="sc", bufs=2, name=f"mbps{h}")
                    nc.tensor.matmul(mbps, lhsT=ones1, rhs=mrow, start=True,
                                     stop=True)
                    mbt = ap_.tile([P, QH], F32, tag="mb", bufs=2, name=f"mb{h}")
                    nc.vector.tensor_copy(mbt, mbps)
                    mb[h] = mbt

                def exp_h(h):
                    for kt in range(NKT):
                        tmp = ap_.tile([P, QH], F32, tag="tmp", bufs=4,
                                       name=f"tmp{h}_{kt}")
                        nc.vector.tensor_sub(tmp, st_tiles[h][kt], mb[h])
                        pt = ap_.tile([P, QH], F16, tag="pt", bufs=36,
                                      name=f"pt{h}_{kt}")
                        nc.scalar.activation(pt, tmp, AF.Exp, scale=SCALE)
                        pt_tiles[h][kt] = pt
                        st_tiles[h][kt] = None

                def av(h):
                    o = [aps.tile([P, 512], F32, tag="avo", bufs=4,
                                  name=f"avo{h}_{m}_{dvc}")
                         for m in range(QH // P) for dvc in range(2)]
                    sm = [aps.tile([P, 1], F32, tag="avs", bufs=2,
                                   name=f"avs{h}_{m}")
                          for m in range(QH // P)]
                    vgt = None
                    for kt in range(NKT):
                        r, k = divmod(kt, SK // P)
                        if k == 0:
                            # batched 1MB V load per rank, on the scalar HWDGE
                            # queue to keep the sync queue free for ktg
                            vgt = ap_.tile([P, DP, SK], F16, tag="vg", bufs=3,
                                           name=f"vg{h}_{r}")
                            nc.scalar.dma_start(
                                vgt, ag_out_v[r].rearrange("d p s -> p d s"))
                        first, last = (kt == 0), (kt == NKT - 1)
                        for m in range(QH // P):
                            lhs = pt_tiles[h][kt][:, m * P:(m + 1) * P]
                            nc.tensor.matmul(o[2 * m], lhsT=lhs,
                                             rhs=vgt[:, 2 * k, :],
                                             start=first, stop=last)
                            nc.tensor.matmul(o[2 * m + 1], lhsT=lhs,
                                             rhs=vgt[:, 2 * k + 1, :],
                                             start=first, stop=last)
                            nc.tensor.matmul(sm[m], lhsT=lhs, rhs=ones16,
                                             start=first, stop=last)
                    for m in range(QH // P):
                        smc = ap_.tile([P, 1], F32, tag="smc", bufs=2,
                                       name=f"smc{h}_{m}")
                        nc.vector.tensor_copy(smc, sm[m])
                        rec = ap_.tile([P, 1], F32, tag="rec", bufs=2,
                                       name=f"rec{h}_{m}")
                        nc.vector.reciprocal(rec, smc)
                        ob = ap_.tile([P, D], F32, tag="ob", bufs=2,
                                      name=f"ob{h}_{m}")
                        nc.vector.tensor_scalar_mul(ob[:, 0:512], o[2 * m], rec)
                        nc.vector.tensor_scalar_mul(ob[:, 512:1024], o[2 * m + 1],
                                                    rec)
                        row0 = h * QH + m * P
                        nc.sync.dma_start(out[row0:row0 + P, :], ob)

                # emission order chosen so PE never idles on softmax:
                scores(0)
                soft_prep(0)
                exp_h(0)
                scores(1)
                soft_prep(1)
                exp_h(1)
                av(0)
                av(1)

    nc.compile()
    return nc


def kernel(x_1, x_2, W_query, W_key, W_value):
    global _CACHED_NC
    if _CACHED_NC is None:
        _CACHED_NC = build_nc()
    nc = _CACHED_NC
    x_1 = np.ascontiguousarray(np.asarray(x_1, dtype=np.float32))
    x_2 = np.ascontiguousarray(np.asarray(x_2, dtype=np.float32))
    wq = np.ascontiguousarray(np.asarray(W_query, dtype=np.float32))
    wk = np.ascontiguousarray(np.asarray(W_key, dtype=np.float32))
    wv = np.ascontiguousarray(np.asarray(W_value, dtype=np.float32))
    in_maps = [{
        "x1s": x_1[c * SQ:(c + 1) * SQ],
        "x2s": x_2[c * SK:(c + 1) * SK],
        "wq": wq, "wk": wk, "wv": wv,
    } for c in range(NCORES)]
    res = run_bass_kernel_spmd(nc, in_maps, core_ids=list(range(NCORES)))
    return np.concatenate([res.results[c]["out"] for c in range(NCORES)], axis=0)


if __name__ == "__main__":
    rng = np.random.default_rng(0)
    x1 = rng.standard_normal((S, D), dtype=np.float32)
    x2 = rng.standard_normal((S, D), dtype=np.float32)
    Wq = rng.random((D, D), dtype=np.float32)
    Wk = rng.random((D, D), dtype=np.float32)
    Wv = rng.random((D, D), dtype=np.float32)
    got = kernel(x_1=x1, x_2=x2, W_query=Wq, W_key=Wk, W_value=Wv)
    q = x1 @ Wq
    k = x2 @ Wk
    v = x2 @ Wv
    s = (q @ k.T) * np.float32(SCALE)
    s -= s.max(-1, keepdims=True)
    p = np.exp(s)
    p /= p.sum(-1, keepdims=True)
    exp = p @ v
    rel = np.linalg.norm(got - exp) / np.linalg.norm(exp)
    print("self-test rel err:", rel)



# revision 5
# speedup vs baseline: 1.4155x; 1.4155x over previous
"""Single-head cross-attention kernel for Trainium2, sharded across 8 NeuronCores.

v2 design (per core c, query+key shard = rows [512c, 512c+512)):
  - x splits into f16 hi + bf16 lo halves, transposed ON-CHIP via PE
    (identity matmul -> PSUM -> ACT/DVE evict). No DRAM round trip: the v1
    xbar-transpose chain kept the PE idle for ~70us at the start.
  - Projections 2-pass (hi f16 + lo bf16 vs f16 W, mixed-dtype matmul),
    fp32 PSUM accumulate. KT/QT produced transposed [d, seq]; V natural.
  - AllGather K first (gates scores), then V (gates AV) - the two AGs
    serialize on the cc stream (~60us each measured), so K's shard is
    evicted as early as possible and V's eviction merely has to beat
    AG-K's completion.
  - PE emission order keeps TensorE busy end-to-end: warmup, T(x2), KT,
    T(x1), QT, V(2-pass), scores, AV.
  - KT/V gathered into SBUF ONCE (8MB + 8MB resident) in p-major layout
    (contiguous per-partition reads), shared by both query halves.
  - Softmax/AV pipeline as v1: scores transposed [keys, q], DVE max chain,
    PE-transpose cross-partition max, exp((S-max)*scale) in f16 = AV lhsT,
    row sums via ones-vector matmul, 1/rowsum on PSUM eviction.

Numerics (host-validated vs fp64): rel err ~1.1e-3, argmax flips 2/4096.
The x-lo correction pass is required (dropping it -> 10 flips, 1.5e-2).
"""
import numpy as np

import concourse.bacc as bacc
import concourse.mybir as mybir
import concourse.tile as tile
from concourse.bass_utils import run_bass_kernel_spmd
from concourse.masks import make_identity

P = 128
D = 1024            # d_in = d_kq = d_v
DP = D // P         # 8 partition tiles of the feature dim
S = 4096            # full sequence length (both x_1 and x_2)
NCORES = 8
SQ = S // NCORES    # 512 query rows per core
SK = S // NCORES    # 512 key rows per core
MT = SQ // P        # 4 row tiles per shard
KT4 = SK // P       # 4 key tiles per rank
NH = 2              # process queries in halves for SBUF + pipelining
QH = SQ // NH       # 256
NKT = S // P        # 32 key tiles of 128
SCALE = float(1.0 / np.sqrt(np.float32(D)))  # 0.03125 exactly

F32 = mybir.dt.float32
F16 = mybir.dt.float16
BF16 = mybir.dt.bfloat16
AX = mybir.AxisListType
AF = mybir.ActivationFunctionType

# lo-correction pass uses f16 W lhsT against bf16 x-lo rhs directly
# (mixed-dtype matmul). If the HW probe fails, set False to fall back to
# on-chip bf16 W copies.
MIXED_LO = True

_CACHED_NC = None


def build_nc():
    nc = bacc.Bacc("TRN2", target_bir_lowering=False, debug=False,
                   num_devices=NCORES)
    x1 = nc.dram_tensor("x1s", [SQ, D], F32, kind="ExternalInput").ap()
    x2 = nc.dram_tensor("x2s", [SK, D], F32, kind="ExternalInput").ap()
    wq = nc.dram_tensor("wq", [D, D], F32, kind="ExternalInput").ap()
    wk = nc.dram_tensor("wk", [D, D], F32, kind="ExternalInput").ap()
    wv = nc.dram_tensor("wv", [D, D], F32, kind="ExternalInput").ap()
    out = nc.dram_tensor("out", [SQ, D], F32, kind="ExternalOutput").ap()

    with tile.TileContext(nc) as tc:
        with tc.tile_pool(name="long", bufs=1) as lp, \
             tc.tile_pool(name="dram", bufs=1, space="DRAM") as dram:
            ident16 = lp.tile([P, P], F16, name="ident16")
            make_identity(nc, ident16)
            identbf = lp.tile([P, P], BF16, name="identbf")
            make_identity(nc, identbf)
            ident32 = lp.tile([P, P], F32, name="ident32")
            make_identity(nc, ident32)
            ones1 = lp.tile([1, P], F32, name="ones1")
            nc.vector.memset(ones1, 1.0)
            ones16 = lp.tile([P, 1], F16, name="ones16")
            nc.vector.memset(ones16, 1.0)
            qt16 = lp.tile([P, DP, SQ], F16, name="qt16")

            # p-major AG layouts: strided 1KB writes, fully contiguous
            # 8KB-per-partition reads on the (8x bigger) gather side.
            ag_in_k = dram.tile([P, DP, SK], F16, name="ag_in_k")
            ag_out_k = dram.tile([NCORES, P, DP, SK], F16,
                                 addr_space="Shared", name="ag_out_k")
            ag_in_v = dram.tile([P, KT4, 2, 512], F16, name="ag_in_v")
            ag_out_v = dram.tile([NCORES, P, KT4, 2, 512], F16,
                                 addr_space="Shared", name="ag_out_v")

            with tc.tile_pool(name="fe", bufs=1) as fe, \
                 tc.tile_pool(name="fe_ps", bufs=1, space="PSUM") as fps:
                warm16 = fe.tile([P, 512], F16, name="warm16")
                nc.vector.memset(warm16, 0.0)
                # W cast-DMAs (SWDGE queue) in need-order: K gates AG-K,
                # Q gates scores, V is needed last.
                wk16 = fe.tile([P, DP, D], F16, name="wk16")
                nc.gpsimd.dma_start(wk16, wk.rearrange("(dp p) n -> p dp n", p=P))
                wq16 = fe.tile([P, DP, D], F16, name="wq16")
                nc.gpsimd.dma_start(wq16, wq.rearrange("(dp p) n -> p dp n", p=P))
                wv16 = fe.tile([P, DP, D], F16, name="wv16")
                nc.gpsimd.dma_start(wv16, wv.rearrange("(dp p) n -> p dp n", p=P))
                if MIXED_LO:
                    wklo, wqlo, wvlo = wk16, wq16, wv16
                else:
                    wklo = fe.tile([P, DP, D], BF16, name="wkbf")
                    nc.vector.tensor_copy(wklo, wk16)
                    wqlo = fe.tile([P, DP, D], BF16, name="wqbf")
                    nc.vector.tensor_copy(wqlo, wq16)
                    wvlo = fe.tile([P, DP, D], BF16, name="wvbf")
                    nc.vector.tensor_copy(wvlo, wv16)

                # x loads: x2 on the sync HWDGE queue, x1 on scalar HWDGE
                xf2 = []
                for m in range(MT):
                    t = fe.tile([P, D], F32, tag="xf2", bufs=MT, name=f"xf2_{m}")
                    nc.sync.dma_start(t, x2[m * P:(m + 1) * P, :])
                    xf2.append(t)
                xf1 = []
                for m in range(MT):
                    t = fe.tile([P, D], F32, tag="xf1", bufs=MT, name=f"xf1_{m}")
                    nc.scalar.dma_start(t, x1[m * P:(m + 1) * P, :])
                    xf1.append(t)

                # PE warm-up: zero-dependency matmuls at t~0 get the HAM
                # un-throttle window counting immediately.
                for w in range(12):
                    wps = fps.tile([P, 512], F32, tag="pp", bufs=3,
                                   name=f"warm{w}")
                    nc.tensor.matmul(wps, lhsT=ident16, rhs=warm16,
                                     start=True, stop=True)

                def split_transpose(xf, hi_t, lo_t, name):
                    for m in range(MT):
                        hi = fe.tile([P, D], F16, tag="xhi", bufs=4,
                                     name=f"{name}_hi{m}")
                        nc.scalar.copy(hi, xf[m])
                        lo = fe.tile([P, D], BF16, tag="xlo", bufs=4,
                                     name=f"{name}_lo{m}")
                        nc.vector.tensor_sub(lo, xf[m], hi)
                        for d in range(DP):
                            tp = fps.tile([P, P], F16, tag="tp16", bufs=2,
                                          name=f"{name}_tp{m}_{d}")
                            nc.tensor.transpose(tp, hi[:, d * P:(d + 1) * P],
                                                ident16)
                            nc.scalar.copy(hi_t[:, d, m * P:(m + 1) * P], tp)
                            tpl = fps.tile([P, P], BF16, tag="tpbf", bufs=2,
                                           name=f"{name}_tpl{m}_{d}")
                            nc.tensor.transpose(tpl, lo[:, d * P:(d + 1) * P],
                                                identbf)
                            nc.vector.tensor_copy(
                                lo_t[:, d, m * P:(m + 1) * P], tpl)

                x2t_hi = fe.tile([P, DP, SK], F16, name="x2t_hi")
                x2t_lo = fe.tile([P, DP, SK], BF16, name="x2t_lo")
                split_transpose(xf2, x2t_hi, x2t_lo, "x2")

                # KT projection: KT[do] = Wk.T @ x2^T  [P, SK], 2-pass
                for do in range(DP):
                    ps = fps.tile([P, SK], F32, tag="pp", bufs=3,
                                  name=f"ktps{do}")
                    cs = slice(do * P, (do + 1) * P)
                    for ki in range(DP):
                        nc.tensor.matmul(ps, lhsT=wk16[:, ki, cs],
                                         rhs=x2t_hi[:, ki, :],
                                         start=(ki == 0), stop=False)
                    for ki in range(DP):
                        nc.tensor.matmul(ps, lhsT=wklo[:, ki, cs],
                                         rhs=x2t_lo[:, ki, :],
                                         start=False, stop=(ki == DP - 1))
                    kt_t = fe.tile([P, SK], F16, tag="ktt", bufs=3,
                                   name=f"kt16_{do}")
                    nc.scalar.copy(kt_t, ps)
                    nc.sync.dma_start(ag_in_k[:, do, :], kt_t)

                # AG-K dispatched as early as possible: it serializes with
                # AG-V on the cc stream and gates the whole scores phase.
                nc.gpsimd.collective_compute(
                    "AllGather", mybir.AluOpType.bypass,
                    replica_groups=[list(range(NCORES))],
                    ins=[ag_in_k.opt()], outs=[ag_out_k.opt()])

                x1t_hi = fe.tile([P, DP, SQ], F16, name="x1t_hi")
                x1t_lo = fe.tile([P, DP, SQ], BF16, name="x1t_lo")
                split_transpose(xf1, x1t_hi, x1t_lo, "x1")

                # QT projection (2-pass), straight into resident qt16
                for do in range(DP):
                    ps = fps.tile([P, SQ], F32, tag="pp", bufs=3,
                                  name=f"qtps{do}")
                    cs = slice(do * P, (do + 1) * P)
                    for ki in range(DP):
                        nc.tensor.matmul(ps, lhsT=wq16[:, ki, cs],
                                         rhs=x1t_hi[:, ki, :],
                                         start=(ki == 0), stop=False)
                    for ki in range(DP):
                        nc.tensor.matmul(ps, lhsT=wqlo[:, ki, cs],
                                         rhs=x1t_lo[:, ki, :],
                                         start=False, stop=(ki == DP - 1))
                    nc.scalar.copy(qt16[:, do, :], ps)

                # V projection, 2-pass (fills the PE window while AG-K
                # flies; AG-V can't start before AG-K finishes anyway)
                for kt in range(KT4):
                    for dvc in range(2):
                        ps = fps.tile([P, 512], F32, tag="pp", bufs=3,
                                      name=f"vps{kt}_{dvc}")
                        ds_ = slice(dvc * 512, (dvc + 1) * 512)
                        for ki in range(DP):
                            nc.tensor.matmul(
                                ps, lhsT=x2t_hi[:, ki, kt * P:(kt + 1) * P],
                                rhs=wv16[:, ki, ds_],
                                start=(ki == 0), stop=False)
                        for ki in range(DP):
                            nc.tensor.matmul(
                                ps, lhsT=x2t_lo[:, ki, kt * P:(kt + 1) * P],
                                rhs=wvlo[:, ki, ds_],
                                start=False, stop=(ki == DP - 1))
                        v_t = fe.tile([P, 512], F16, tag="vt", bufs=3,
                                      name=f"v16_{kt}_{dvc}")
                        nc.vector.tensor_copy(v_t, ps)
                        nc.sync.dma_start(ag_in_v[:, kt, dvc, :], v_t)

                nc.gpsimd.collective_compute(
                    "AllGather", mybir.AluOpType.bypass,
                    replica_groups=[list(range(NCORES))],
                    ins=[ag_in_v.opt()], outs=[ag_out_v.opt()])

            # ---- attention: scores -> softmax -> AV, in query halves ----
            with tc.tile_pool(name="attn", bufs=1) as ap_, \
                 tc.tile_pool(name="attn_ps", bufs=1, space="PSUM") as aps:
                # resident K^T and V for all 4096 keys, loaded once
                ktg = ap_.tile([P, NCORES, DP, SK], F16, name="ktg")
                for r in range(NCORES):
                    nc.sync.dma_start(ktg[:, r], ag_out_k[r])
                vg = ap_.tile([P, NCORES, KT4, 2, 512], F16, name="vg")
                for r in range(NCORES):
                    nc.gpsimd.dma_start(vg[:, r], ag_out_v[r])

                st_tiles = [[None] * NKT for _ in range(NH)]
                pt_tiles = [[None] * NKT for _ in range(NH)]
                m1 = [None] * NH
                mb = [None] * NH

                def scores(h):
                    qsl = slice(h * QH, (h + 1) * QH)
                    for kt in range(NKT):
                        r, k = divmod(kt, KT4)
                        ps = aps.tile([P, QH], F32, tag="sc", bufs=2,
                                      name=f"stps{h}_{kt}")
                        for d in range(DP):
                            nc.tensor.matmul(
                                ps, lhsT=ktg[:, r, d, k * P:(k + 1) * P],
                                rhs=qt16[:, d, qsl],
                                start=(d == 0), stop=(d == DP - 1))
                        st = ap_.tile([P, QH], F32, tag="st", bufs=34,
                                      name=f"st{h}_{kt}")
                        nc.vector.tensor_copy(st, ps)
                        st_tiles[h][kt] = st
                        mn = ap_.tile([P, QH], F32, tag="m1", bufs=2,
                                      name=f"m1_{h}_{kt}")
                        if kt == 0:
                            nc.vector.tensor_copy(mn, st)
                        else:
                            nc.vector.tensor_max(mn, m1[h], st)
                        m1[h] = mn

                def soft_prep(h):
                    # cross-partition max: PE-transpose m1 128-blocks, DVE
                    # reduce, broadcast back with a rank-1 matmul
                    mrow = ap_.tile([1, QH], F32, tag="mrow", bufs=2,
                                    name=f"mrow{h}")
                    for b in range(QH // P):
                        tps = aps.tile([P, P], F32, tag="sc", bufs=2,
                                       name=f"tps{h}_{b}")
                        nc.tensor.transpose(tps, m1[h][:, b * P:(b + 1) * P],
                                            ident32)
                        mq = ap_.tile([P, 1], F32, tag="mq", bufs=2,
                                      name=f"mq{h}_{b}")
                        nc.vector.reduce_max(mq, tps, axis=AX.X)
                        rps = aps.tile([1, P], F32, tag="sc", bufs=2,
                                       name=f"rps{h}_{b}")
                        nc.tensor.transpose(rps, mq, ident32)
                        nc.vector.tensor_copy(mrow[:, b * P:(b + 1) * P], rps)
                    mbps = aps.tile([P, QH], F32, tag="sc", bufs=2,
                                    name=f"mbps{h}")
                    nc.tensor.matmul(mbps, lhsT=ones1, rhs=mrow, start=True,
                                     stop=True)
                    mbt = ap_.tile([P, QH], F32, tag="mb", bufs=2,
                                   name=f"mb{h}")
                    nc.vector.tensor_copy(mbt, mbps)
                    mb[h] = mbt

                def exp_h(h):
                    for kt in range(NKT):
                        tmp = ap_.tile([P, QH], F32, tag="tmp", bufs=3,
                                       name=f"tmp{h}_{kt}")
                        nc.vector.tensor_sub(tmp, st_tiles[h][kt], mb[h])
                        pt = ap_.tile([P, QH], F16, tag="pt", bufs=34,
                                      name=f"pt{h}_{kt}")
                        nc.scalar.activation(pt, tmp, AF.Exp, scale=SCALE)
                        pt_tiles[h][kt] = pt
                        st_tiles[h][kt] = None

                def av(h):
                    o = [aps.tile([P, 512], F32, tag="avo", bufs=4,
                                  name=f"avo{h}_{m}_{dvc}")
                         for m in range(QH // P) for dvc in range(2)]
                    sm = [aps.tile([P, 1], F32, tag="avs", bufs=2,
                                   name=f"avs{h}_{m}")
                          for m in range(QH // P)]
                    for kt in range(NKT):
                        r, k = divmod(kt, KT4)
                        first, last = (kt == 0), (kt == NKT - 1)
                        for m in range(QH // P):
                            lhs = pt_tiles[h][kt][:, m * P:(m + 1) * P]
                            nc.tensor.matmul(o[2 * m], lhsT=lhs,
                                             rhs=vg[:, r, k, 0, :],
                                             start=first, stop=last)
                            nc.tensor.matmul(o[2 * m + 1], lhsT=lhs,
                                             rhs=vg[:, r, k, 1, :],
                                             start=first, stop=last)
                            nc.tensor.matmul(sm[m], lhsT=lhs, rhs=ones16,
                                             start=first, stop=last)
                    for m in range(QH // P):
                        smc = ap_.tile([P, 1], F32, tag="smc", bufs=2,
                                       name=f"smc{h}_{m}")
                        nc.vector.tensor_copy(smc, sm[m])
                        rec = ap_.tile([P, 1], F32, tag="rec", bufs=2,
                                       name=f"rec{h}_{m}")
                        nc.vector.reciprocal(rec, smc)
                        ob = ap_.tile([P, D], F32, tag="ob", bufs=2,
                                      name=f"ob{h}_{m}")
                        nc.vector.tensor_scalar_mul(ob[:, 0:512], o[2 * m], rec)
                        nc.vector.tensor_scalar_mul(ob[:, 512:1024],
                                                    o[2 * m + 1], rec)
                        row0 = h * QH + m * P
                        nc.sync.dma_start(out[row0:row0 + P, :], ob)

                # emission order chosen so PE never idles on softmax:
                scores(0)
                soft_prep(0)
                exp_h(0)
                scores(1)
                soft_prep(1)
                exp_h(1)
                av(0)
                av(1)

    nc.compile()
    return nc


def kernel(x_1, x_2, W_query, W_key, W_value):
    global _CACHED_NC
    if _CACHED_NC is None:
        _CACHED_NC = build_nc()
    nc = _CACHED_NC
    x_1 = np.ascontiguousarray(np.asarray(x_1, dtype=np.float32))
    x_2 = np.ascontiguousarray(np.asarray(x_2, dtype=np.float32))
    wq = np.ascontiguousarray(np.asarray(W_query, dtype=np.float32))
    wk = np.ascontiguousarray(np.asarray(W_key, dtype=np.float32))
    wv = np.ascontiguousarray(np.asarray(W_value, dtype=np.float32))
    in_maps = [{
        "x1s": x_1[c * SQ:(c + 1) * SQ],
        "x2s": x_2[c * SK:(c + 1) * SK],
        "wq": wq, "wk": wk, "wv": wv,
    } for c in range(NCORES)]
    res = run_bass_kernel_spmd(nc, in_maps, core_ids=list(range(NCORES)))
    return np.concatenate([res.results[c]["out"] for c in range(NCORES)], axis=0)


if __name__ == "__main__":
    rng = np.random.default_rng(0)
    x1 = rng.standard_normal((S, D), dtype=np.float32)
    x2 = rng.standard_normal((S, D), dtype=np.float32)
    Wq = rng.random((D, D), dtype=np.float32)
    Wk = rng.random((D, D), dtype=np.float32)
    Wv = rng.random((D, D), dtype=np.float32)
    got = kernel(x_1=x1, x_2=x2, W_query=Wq, W_key=Wk, W_value=Wv)
    q = x1 @ Wq
    k = x2 @ Wk
    v = x2 @ Wv
    s = (q @ k.T) * np.float32(SCALE)
    s -= s.max(-1, keepdims=True)
    p = np.exp(s)
    p /= p.sum(-1, keepdims=True)
    exp = p @ v
    rel = np.linalg.norm(got - exp) / np.linalg.norm(exp)
    print("self-test rel err:", rel)
